# revision 1
# baseline (speedup 1.0000x reference)
"""CQAttention Trainium2 kernel.

Math (per batch b):
  S = (C*w3) @ Q^T + (C@w1)[:,None] + (Q@w2)[None,:] (+bias, dropped: softmax-invariant)
  Sq = softmax over q of qmask-masked S ; Sc = softmax over c of cmask-masked S
  A = Sq@Q ; Bm = Sq @ (Sc^T @ C) ; out = [C | A | C*A | C*Bm]

Device algorithm (no max-subtraction: |S| < 1 so exp is safe; masks become
additive -1e30 terms, i.e. multiplicative exp factors that either cancel in
the normalized ratios or are folded into operands):
  ST   = (Q*w3) @ C^T                       [q, c]   (PE, fp32r)
  E_q  = exp(ST + (rq + qneg)[q])           [q, c]   row-masked
  X    = exp(ST)                            [q, c]
  Cg   = [C|1] * exp(rc + cneg)[c]          [c, d+2] col-mask folded into C
  T1s  = (X^T @ Cg) normalized              [q, d]   == Sc^T @ C
  psA  = E_q^T @ [Q|1]                      [c, d+2] unnormalized A | rowsum
  psB  = E_q^T @ T1s                        [c, d]   unnormalized Bm
  A = psA * rr ; CA = C * A ; CBm = C * psB * rr      (rr = 1/rowsum)

Sharding: data-parallel over batch, 4 batches per core on 8 cores.
"""

import os

import numpy as np

NEG_INF = -1e30
B_FULL, LC, LQ, D = 32, 1024, 128, 256
N_CORES = 8
NB = B_FULL // N_CORES  # batches per core
KC = LC // 128  # c-tiles per batch (8)

_CACHE = {}


def _build_nc():
    import concourse.bacc as bacc
    import concourse.mybir as mybir
    from concourse import tile
    from concourse.masks import make_identity

    fp32 = mybir.dt.float32
    MULT = mybir.AluOpType.mult
    ADD = mybir.AluOpType.add
    EXP = mybir.ActivationFunctionType.Exp

    nc = bacc.Bacc("TRN2", target_bir_lowering=False, debug=False)

    use_r = os.environ.get("FP32R", "1") != "0"
    fp32r = mybir.dt.float32r
    mmdt = fp32r if use_r else fp32

    C_d = nc.dram_tensor("C", [NB, LC, D], fp32, kind="ExternalInput")
    Q_d = nc.dram_tensor("Q", [NB, LQ, D], fp32, kind="ExternalInput")
    cneg_d = nc.dram_tensor("cneg", [NB, 128, KC], fp32, kind="ExternalInput")
    qneg_d = nc.dram_tensor("qneg", [NB, 128, 1], fp32, kind="ExternalInput")
    w_d = nc.dram_tensor("w_pk", [128, 6], fp32, kind="ExternalInput")
    w2bc_d = nc.dram_tensor("w2bc", [128, D], fp32, kind="ExternalInput")
    out_d = nc.dram_tensor("out", [NB, LC, 4 * D], fp32, kind="ExternalOutput")

    with tile.TileContext(nc) as tc:
        with (
            tc.tile_pool(name="const", bufs=1) as const,
            tc.tile_pool(name="cpool", bufs=NB) as p_c,
            tc.tile_pool(name="cgpool", bufs=2) as p_cg,
            tc.tile_pool(name="qpool", bufs=NB) as p_q,
            tc.tile_pool(name="mpool", bufs=NB) as p_m,
            tc.tile_pool(name="ctpool", bufs=2) as p_ct,
            tc.tile_pool(name="qtpool", bufs=2) as p_qt,
            tc.tile_pool(name="epool", bufs=2) as p_e,
            tc.tile_pool(name="opool", bufs=3) as p_o,
            tc.tile_pool(name="smpool", bufs=4) as p_sm,
            tc.tile_pool(name="pst", bufs=2, space="PSUM") as ps_t,
            tc.tile_pool(name="psst", bufs=2, space="PSUM") as ps_st,
            tc.tile_pool(name="pst1", bufs=1, space="PSUM") as ps_t1,
            tc.tile_pool(name="psa", bufs=2, space="PSUM") as ps_a,
            tc.tile_pool(name="psb", bufs=1, space="PSUM") as ps_b,
        ):
            ident = const.tile([128, 128], fp32)
            make_identity(nc, ident)
            w_sb = const.tile([128, 6], fp32)
            nc.sync.dma_start(w_sb, w_d.ap())
            w2bc = const.tile([128, D], fp32)
            nc.sync.dma_start(w2bc, w2bc_d.ap())
            # duplicated-column w1 (fp32r, even-N rhs for the rc matmuls)
            w1r2 = const.tile([128, 2, 2], mmdt, tag="w1r2")
            for dk in range(2):
                for j in range(2):
                    nc.vector.tensor_copy(w1r2[:, dk, j : j + 1], w_sb[:, dk : dk + 1])

            # ---- hoisted input loads for all batches ----
            C1s, Q1s, cnegs, qnegs = [], [], [], []
            for b in range(NB):
                C1 = p_c.tile([128, KC, D + 2], fp32, tag="c")
                nc.vector.memset(C1[:, :, D : D + 2], 1.0)
                nc.sync.dma_start(
                    C1[:, :, 0:D], C_d.ap()[b].rearrange("(k p) d -> p k d", p=128)
                )
                Q1 = p_q.tile([128, D + 2], fp32, tag="q")
                nc.vector.memset(Q1[:, D : D + 2], 1.0)
                nc.sync.dma_start(Q1[:, 0:D], Q_d.ap()[b])
                cneg = p_m.tile([128, KC], fp32, tag="cneg")
                nc.sync.dma_start(cneg, cneg_d.ap()[b])
                qneg = p_m.tile([128, 1], fp32, tag="qneg")
                nc.sync.dma_start(qneg, qneg_d.ap()[b])
                # C segment of the output goes straight from SBUF.
                nc.sync.dma_start(
                    out_d.ap()[b, :, 0:D].rearrange("(k p) d -> p k d", p=128),
                    C1[:, :, 0:D],
                )
                C1s.append(C1)
                Q1s.append(Q1)
                cnegs.append(cneg)
                qnegs.append(qneg)

            for b in range(NB):
                C1, Q1, cneg, qneg = C1s[b], Q1s[b], cnegs[b], qnegs[b]

                # rounded [Q | 1] rhs
                if use_r:
                    Q1r = p_q.tile([128, D + 2], fp32r, tag="qr")
                    nc.vector.tensor_copy(Q1r, Q1)
                else:
                    Q1r = Q1

                # ---- rq = Q@w2 (gpsimd product + DVE reduce) ----
                scr = p_sm.tile([128, D], fp32, tag="ttrs")
                nc.gpsimd.tensor_mul(scr, Q1[:, 0:D], w2bc)
                rq = p_sm.tile([128, 1], fp32, tag="rq")
                nc.vector.tensor_reduce(rq, scr, mybir.AxisListType.X, ADD)
                bias_q = p_sm.tile([128, 1], fp32, tag="biasq")
                nc.vector.tensor_add(bias_q, rq, qneg)

                # ---- QT3 = (Q^T) * w3 per d-chunk ----
                QT3 = p_qt.tile([128, 2, 128], mmdt, tag="qtw3")
                for dk in range(2):
                    pt = ps_t.tile([128, 512], fp32, tag="pt")
                    nc.tensor.transpose(
                        pt[:, 0:128], Q1[:, dk * 128 : (dk + 1) * 128], ident
                    )
                    nc.vector.tensor_scalar_mul(
                        QT3[:, dk], pt[:, 0:128], w_sb[:, 4 + dk : 5 + dk]
                    )

                # ---- CT (transpose C): 4 transposes per PSUM bank, 1 copy ----
                CT = p_ct.tile([128, 2, LC], mmdt, tag="ct")
                for dk in range(2):
                    for h in range(2):
                        pt = ps_t.tile([128, 512], fp32, tag="pt")
                        for j in range(4):
                            k = h * 4 + j
                            nc.tensor.transpose(
                                pt[:, j * 128 : (j + 1) * 128],
                                C1[:, k, dk * 128 : (dk + 1) * 128],
                                ident,
                            )
                        dst = CT[:, dk, h * 512 : (h + 1) * 512]
                        if (dk * 2 + h) % 2 == 0:
                            nc.scalar.copy(dst, pt)
                        else:
                            nc.vector.tensor_copy(dst, pt)

                # ---- rc = C@w1 on PE (tiny fp32r matmuls on CT) ----
                rc_ps = ps_t.tile([128, 2 * KC], fp32, tag="pt")
                for k in range(KC):
                    for dk in range(2):
                        nc.tensor.matmul(
                            rc_ps[:, 2 * k : 2 * k + 2],
                            CT[:, dk, k * 128 : (k + 1) * 128],
                            w1r2[:, dk],
                            start=(dk == 0),
                            stop=(dk == 1),
                        )
                rc_cneg = p_sm.tile([128, KC], fp32, tag="rccneg")
                nc.vector.tensor_add(
                    rc_cneg,
                    rc_ps.rearrange("p (k two) -> p k two", two=2)[:, :, 0],
                    cneg,
                )
                gexp = p_sm.tile([128, KC], fp32, tag="gexp")
                nc.scalar.activation(gexp, rc_cneg, EXP)

                # ---- Cg = [C|1] * exp(rc+cneg): col-mask folded into rhs ----
                Cg = p_cg.tile([128, KC, D + 2], mmdt, tag="cg")
                for k in range(KC):
                    nc.vector.tensor_scalar_mul(Cg[:, k], C1[:, k], gexp[:, k : k + 1])

                # ---- main matmul ST = (Q*w3) @ C^T, then E_q / X ----
                E_q = p_e.tile([128, LC], mmdt, tag="eq")
                X = p_e.tile([128, LC], fp32, tag="x")
                for h in range(2):
                    st = ps_st.tile([128, 512], fp32, tag="st")
                    for dk in range(2):
                        nc.tensor.matmul(
                            st,
                            QT3[:, dk],
                            CT[:, dk, h * 512 : (h + 1) * 512],
                            start=(dk == 0),
                            stop=(dk == 1),
                        )
                    nc.scalar.activation(
                        E_q[:, h * 512 : (h + 1) * 512], st, EXP, bias=bias_q
                    )
                    nc.scalar.activation(X[:, h * 512 : (h + 1) * 512], st, EXP)

                # ---- XT = X^T (raw; mask/rc factors live in Cg) ----
                XT = p_e.tile([128, KC, 128], mmdt, tag="xt")
                XTflat = XT.rearrange("p k q -> p (k q)")
                for h in range(2):
                    pt = ps_t.tile([128, 512], fp32, tag="pt")
                    for j in range(4):
                        k = h * 4 + j
                        nc.tensor.transpose(
                            pt[:, j * 128 : (j + 1) * 128],
                            X[:, k * 128 : (k + 1) * 128],
                            ident,
                        )
                    dst = XTflat[:, h * 512 : (h + 1) * 512]
                    if h % 2 == 0:
                        nc.scalar.copy(dst, pt)
                    else:
                        nc.vector.tensor_copy(dst, pt)

                # ---- T1s = (Sc^T C) = (X^T @ Cg) normalized ----
                t1 = ps_t1.tile([128, D + 2], fp32, tag="t1")
                for k in range(KC):
                    nc.tensor.matmul(
                        t1,
                        XT[:, k],
                        Cg[:, k],
                        start=(k == 0),
                        stop=(k == KC - 1),
                    )
                recipT = p_sm.tile([128, 1], fp32, tag="recipT")
                nc.vector.reciprocal(recipT, t1[:, D : D + 1])
                T1s = p_sm.tile([128, D], mmdt, tag="t1s")
                nc.vector.tensor_scalar_mul(T1s, t1[:, 0:D], recipT)

                # ---- per c-tile: A / CA / CBm (stores paired over 2 tiles) ----
                for k in range(KC):
                    kk = k % 2
                    if kk == 0:
                        osb = p_o.tile([128, 2, 3 * D], fp32, tag="osb")
                    eq_k = E_q[:, k * 128 : (k + 1) * 128]
                    psA = ps_a.tile([128, D + 2], fp32, tag="psa")
                    nc.tensor.matmul(psA, eq_k, Q1r[:], start=True, stop=True)
                    psB = ps_b.tile([128, D], fp32, tag="psb")
                    nc.tensor.matmul(psB, eq_k, T1s[:], start=True, stop=True)

                    rr = p_sm.tile([128, 1], fp32, tag="rr")
                    nc.vector.reciprocal(rr, psA[:, D : D + 1])

                    # A = psA * rr  (ACT, per-partition scale)
                    nc.scalar.mul(osb[:, kk, 0:D], psA[:, 0:D], rr)
                    # CA = C * A  (GPSIMD, reads the extracted A)
                    nc.gpsimd.tensor_mul(
                        osb[:, kk, D : 2 * D], C1[:, k, 0:D], osb[:, kk, 0:D]
                    )
                    # CBm = (psB * rr) * C  (DVE fused)
                    nc.vector.scalar_tensor_tensor(
                        osb[:, kk, 2 * D : 3 * D], psB, rr, C1[:, k, 0:D], MULT, MULT
                    )
                    if kk == 1:
                        nc.sync.dma_start(
                            out_d.ap()[
                                b, (k - 1) * 128 : (k + 1) * 128, D : 4 * D
                            ].rearrange("(k p) n -> p k n", p=128),
                            osb,
                        )

    nc.compile()
    return nc


def _get_nc():
    if "nc" not in _CACHE:
        _CACHE["nc"] = _build_nc()
    return _CACHE["nc"]


def _make_in_maps(C, Q, cmask, qmask, Wo_w):
    C = np.ascontiguousarray(C, dtype=np.float32)
    Q = np.ascontiguousarray(Q, dtype=np.float32)
    cneg = ((1.0 - cmask.astype(np.float32)) * NEG_INF).astype(np.float32)
    qneg = ((1.0 - qmask.astype(np.float32)) * NEG_INF).astype(np.float32)
    cneg = np.ascontiguousarray(cneg.reshape(B_FULL, KC, 128).transpose(0, 2, 1))
    qneg = np.ascontiguousarray(qneg.reshape(B_FULL, 128, 1))
    Wo_w = Wo_w.astype(np.float32)
    w_pk = np.ascontiguousarray(Wo_w.reshape(6, 128).T)
    w2bc = np.ascontiguousarray(np.broadcast_to(Wo_w[D : 2 * D], (128, D)))
    in_maps = []
    for i in range(N_CORES):
        sl = slice(i * NB, (i + 1) * NB)
        in_maps.append(
            {
                "C": np.ascontiguousarray(C[sl]),
                "Q": np.ascontiguousarray(Q[sl]),
                "cneg": np.ascontiguousarray(cneg[sl]),
                "qneg": np.ascontiguousarray(qneg[sl]),
                "w_pk": w_pk,
                "w2bc": w2bc,
            }
        )
    return in_maps


def kernel(C, Q, cmask, qmask, Wo_w, Wo_b):
    from concourse.bass_utils import run_bass_kernel_spmd

    nc = _get_nc()
    in_maps = _make_in_maps(C, Q, cmask, qmask, Wo_w)
    res = run_bass_kernel_spmd(nc, in_maps, core_ids=list(range(N_CORES)))
    out = np.concatenate([res.results[i]["out"] for i in range(N_CORES)], axis=0)
    return out



# revision 13
# speedup vs baseline: 1.2572x; 1.2572x over previous
"""CQAttention Trainium2 kernel.

Math (per batch b):
  S = (C*w3) @ Q^T + (C@w1)[:,None] + (Q@w2)[None,:] (+bias, dropped: softmax-invariant)
  Sq = softmax over q of qmask-masked S ; Sc = softmax over c of cmask-masked S
  A = Sq@Q ; Bm = Sq @ (Sc^T @ C) ; out = [C | A | C*A | C*Bm]

Device algorithm. No max-subtraction (|S| < 1 so exp is safe). All rank-1
bias/mask factors are precomputed on host as multiplicative exp factors and
folded into matmul operands, so the device only computes:
  ST  = (Q*w3) @ C^T                [q, c]   (PE, fp32r)
  X   = exp(ST)                     [q, c]
  XTg = X^T * g[c]                  [c, q]   g = exp(C@w1 + cneg), fold in the
                                             PSUM->SBUF copy of the transpose
  T1  = XTg^T @ [C|1]               [q, d+2] numerator | W (col denominators)
  T1g = T1[:, :d] * (eb/W)[q]       [q, d]   eb = exp(Q@w2 + qneg)
  psA = X^T @ [Q*eb | eb]           [c, d+2] unnormalized A | Z (row denoms)
  psB = X^T @ T1g                   [c, d]   unnormalized Bm
  A = psA/Z ; CA = C*A ; CBm = C*psB/Z

Sharding: data-parallel over batch, 4 batches per core on 8 cores.
"""

import numpy as np

NEG_INF = -1e30
B_FULL, LC, LQ, D = 32, 1024, 128, 256
D2 = D + 2
N_CORES = 8
NB = B_FULL // N_CORES  # batches per core
KC = LC // 128  # c-tiles per batch (8)

_CACHE = {}


def _build_nc():
    import concourse.bacc as bacc
    import concourse.mybir as mybir
    from concourse import tile
    from concourse.masks import make_identity

    fp32 = mybir.dt.float32
    fp32r = mybir.dt.float32r
    MULT = mybir.AluOpType.mult
    EXP = mybir.ActivationFunctionType.Exp

    nc = bacc.Bacc("TRN2", target_bir_lowering=False, debug=False)

    C_d = nc.dram_tensor("Cp", [NB, LC, D2], fp32, kind="ExternalInput")
    qt3_d = nc.dram_tensor("qt3", [128, NB, 2, 128], fp32, kind="ExternalInput")
    qg_d = nc.dram_tensor("qg", [128, NB, D2], fp32, kind="ExternalInput")
    meta_d = nc.dram_tensor("meta", [128, NB, KC + 1], fp32, kind="ExternalInput")
    out_d = nc.dram_tensor("out", [NB, LC, 4 * D], fp32, kind="ExternalOutput")

    with tile.TileContext(nc) as tc:
        with (
            tc.tile_pool(name="const", bufs=1) as const,
            tc.tile_pool(name="cpool", bufs=NB) as p_c,
            tc.tile_pool(name="crpool", bufs=2) as p_cr,
            tc.tile_pool(name="ctpool", bufs=2) as p_ct,
            tc.tile_pool(name="xpool", bufs=2) as p_x,
            tc.tile_pool(name="xtpool", bufs=2) as p_xt,
            tc.tile_pool(name="t1gpool", bufs=2) as p_t1g,
            tc.tile_pool(name="smpool", bufs=6) as p_sm,
            tc.tile_pool(name="opool", bufs=6) as p_o,
            tc.tile_pool(name="pstr", bufs=3, space="PSUM") as ps_tr,
            tc.tile_pool(name="pst1", bufs=1, space="PSUM") as ps_t1,
            tc.tile_pool(name="psa", bufs=2, space="PSUM") as ps_a,
            tc.tile_pool(name="psb", bufs=2, space="PSUM") as ps_b,
        ):
            ident = const.tile([128, 128], fp32)
            make_identity(nc, ident)
            identr = const.tile([128, 128], fp32r)
            nc.vector.tensor_copy(identr, ident)

            qt3f = const.tile([128, NB, 2, 128], fp32)
            nc.sync.dma_start(qt3f, qt3_d.ap())
            qgf = const.tile([128, NB, D2], fp32)
            nc.sync.dma_start(qgf, qg_d.ap())
            meta = const.tile([128, NB, KC + 1], fp32)
            nc.sync.dma_start(meta, meta_d.ap())
            # one-time fp32r rounding of the shared matmul operands
            qt3 = const.tile([128, NB, 2, 128], fp32r)
            nc.vector.tensor_copy(qt3, qt3f)
            qg = const.tile([128, NB, D2], fp32r)
            nc.scalar.copy(qg, qgf)

            # ---- hoisted input loads + C passthrough stores ----
            C1s = []
            for b in range(NB):
                C1 = p_c.tile([128, KC, D2], fp32, tag="c")
                nc.sync.dma_start(
                    C1, C_d.ap()[b].rearrange("(k p) d -> p k d", p=128)
                )
                C1s.append(C1)
            for b in range(NB):
                nc.sync.dma_start(
                    out_d.ap()[b, :, 0:D].rearrange("(k p) d -> p k d", p=128),
                    C1s[b][:, :, 0:D],
                )

            for b in range(NB):
                C1 = C1s[b]

                # ---- rounded [C|1] copy for the T1 rhs ----
                Cr = p_cr.tile([128, KC, D2], fp32r, tag="cr")
                nc.scalar.copy(Cr[:, 0 : KC // 2], C1[:, 0 : KC // 2])
                nc.vector.tensor_copy(Cr[:, KC // 2 : KC], C1[:, KC // 2 : KC])

                # ---- CT = C^T per d-chunk ----
                CT = p_ct.tile([128, 2, LC], fp32r, tag="ct")
                for dk in range(2):
                    for h in range(2):
                        pt = ps_tr.tile([128, 512], fp32, tag="pt")
                        for j in range(4):
                            k = h * 4 + j
                            nc.tensor.transpose(
                                pt[:, j * 128 : (j + 1) * 128],
                                C1[:, k, dk * 128 : (dk + 1) * 128],
                                ident,
                            )
                        dst = CT[:, dk, h * 512 : (h + 1) * 512]
                        if (dk * 2 + h) % 2 == 0:
                            nc.scalar.copy(dst, pt)
                        else:
                            nc.vector.tensor_copy(dst, pt)

                # ---- ST = (Q*w3) @ C^T ; X = exp(ST) ----
                X = p_x.tile([128, LC], fp32r, tag="x")
                for h in range(2):
                    st = ps_tr.tile([128, 512], fp32, tag="pt")
                    for dk in range(2):
                        nc.tensor.matmul(
                            st,
                            qt3[:, b, dk],
                            CT[:, dk, h * 512 : (h + 1) * 512],
                            start=(dk == 0),
                            stop=(dk == 1),
                        )
                    nc.scalar.activation(X[:, h * 512 : (h + 1) * 512], st, EXP)

                # ---- XTg = X^T * g (g folded into the PSUM->SBUF copy) ----
                XT = p_xt.tile([128, KC, 128], fp32r, tag="xt")
                for h in range(2):
                    pt = ps_tr.tile([128, 512], fp32r, tag="pt")
                    for j in range(4):
                        k = h * 4 + j
                        nc.tensor.transpose(
                            pt[:, j * 128 : (j + 1) * 128],
                            X[:, k * 128 : (k + 1) * 128],
                            identr,
                        )
                    for j in range(4):
                        k = h * 4 + j
                        src = pt[:, j * 128 : (j + 1) * 128]
                        gk = meta[:, b, k : k + 1]
                        if j % 2 == 0:
                            nc.scalar.mul(XT[:, k], src, gk)
                        else:
                            nc.vector.tensor_scalar_mul(XT[:, k], src, gk)

                # ---- T1 = XTg^T @ [C|1] ; T1g = T1 * (eb/W) ----
                t1 = ps_t1.tile([128, D2], fp32, tag="t1")
                for k in range(KC):
                    nc.tensor.matmul(
                        t1,
                        XT[:, k],
                        Cr[:, k],
                        start=(k == 0),
                        stop=(k == KC - 1),
                    )
                recipT = p_sm.tile([128, 1], fp32, tag="recipT")
                nc.vector.reciprocal(recipT, t1[:, D : D + 1])
                scal = p_sm.tile([128, 1], fp32, tag="scal")
                nc.vector.tensor_mul(scal, recipT, meta[:, b, KC : KC + 1])
                T1g = p_t1g.tile([128, D], fp32r, tag="t1g")
                nc.vector.tensor_scalar_mul(T1g, t1[:, 0:D], scal)

                # ---- per c-tile: psA / psB -> A / CA / CBm ----
                psAs = [None] * KC
                psAs[0] = ps_a.tile([128, D2], fp32, tag="psa", name="psa0")
                nc.tensor.matmul(psAs[0], X[:, 0:128], qg[:, b], start=True, stop=True)
                for k in range(KC):
                    kk = k % 2
                    if kk == 0:
                        osb = p_o.tile([128, 2, 3 * D], fp32, tag="osb")
                    if k + 1 < KC:
                        psAs[k + 1] = ps_a.tile([128, D2], fp32, tag="psa", name="psa")
                        nc.tensor.matmul(
                            psAs[k + 1],
                            X[:, (k + 1) * 128 : (k + 2) * 128],
                            qg[:, b],
                            start=True,
                            stop=True,
                        )
                    psA = psAs[k]
                    psB = ps_b.tile([128, D], fp32, tag="psb")
                    nc.tensor.matmul(
                        psB, X[:, k * 128 : (k + 1) * 128], T1g, start=True, stop=True
                    )

                    rr = p_sm.tile([128, 1], fp32, tag="rr")
                    nc.vector.reciprocal(rr, psA[:, D : D + 1])

                    # A = psA * rr  (ACT, per-partition scale)
                    nc.scalar.mul(osb[:, kk, 0:D], psA[:, 0:D], rr)
                    # CA = C * A  (GPSIMD)
                    nc.gpsimd.tensor_mul(
                        osb[:, kk, D : 2 * D], C1[:, k, 0:D], osb[:, kk, 0:D]
                    )
                    # CBm = (psB * rr) * C  (DVE fused)
                    nc.vector.scalar_tensor_tensor(
                        osb[:, kk, 2 * D : 3 * D], psB, rr, C1[:, k, 0:D], MULT, MULT
                    )
                    if kk == 1:
                        nc.sync.dma_start(
                            out_d.ap()[
                                b, (k - 1) * 128 : (k + 1) * 128, D : 4 * D
                            ].rearrange("(k p) n -> p k n", p=128),
                            osb,
                        )

    nc.compile()
    return nc


def _get_nc():
    if "nc" not in _CACHE:
        _CACHE["nc"] = _build_nc()
    return _CACHE["nc"]


def _make_in_maps(C, Q, cmask, qmask, Wo_w):
    C = np.ascontiguousarray(C, dtype=np.float32)
    Q = np.ascontiguousarray(Q, dtype=np.float32)
    w = np.asarray(Wo_w, dtype=np.float32)
    w1, w2, w3 = w[:D], w[D : 2 * D], w[2 * D :]

    rc = (C @ w1).astype(np.float32)  # [B, Lc]
    rq = (Q @ w2).astype(np.float32)  # [B, Lq]
    cneg = ((1.0 - cmask.astype(np.float32)) * NEG_INF).astype(np.float32)
    qneg = ((1.0 - qmask.astype(np.float32)) * NEG_INF).astype(np.float32)
    with np.errstate(under="ignore", over="ignore"):
        g = np.exp(rc + cneg).astype(np.float32)  # [B, Lc]
        eb = np.exp(rq + qneg).astype(np.float32)  # [B, Lq]

    ones2 = np.ones((B_FULL, LC, 2), np.float32)
    Cp = np.concatenate([C, ones2], axis=2)  # [B, Lc, 258]

    QT3 = (C.dtype.type(1) * Q.transpose(0, 2, 1) * w3[None, :, None]).reshape(
        B_FULL, 2, 128, LQ
    )
    QT3 = QT3.transpose(0, 2, 1, 3)  # [B, 128(p), 2(dk), 128(q)]

    ebc = eb[:, :, None]
    Qg = np.concatenate([Q * ebc, ebc, ebc], axis=2)  # [B, 128, 258]

    gm = g.reshape(B_FULL, KC, 128).transpose(0, 2, 1)  # [B, 128, KC]
    meta = np.concatenate([gm, eb[:, :, None]], axis=2)  # [B, 128, KC+1]

    in_maps = []
    for i in range(N_CORES):
        sl = slice(i * NB, (i + 1) * NB)
        in_maps.append(
            {
                "Cp": np.ascontiguousarray(Cp[sl]),
                "qt3": np.ascontiguousarray(QT3[sl].transpose(1, 0, 2, 3)),
                "qg": np.ascontiguousarray(Qg[sl].transpose(1, 0, 2)),
                "meta": np.ascontiguousarray(meta[sl].transpose(1, 0, 2)),
            }
        )
    return in_maps


def kernel(C, Q, cmask, qmask, Wo_w, Wo_b):
    from concourse.bass_utils import run_bass_kernel_spmd

    nc = _get_nc()
    in_maps = _make_in_maps(C, Q, cmask, qmask, Wo_w)
    res = run_bass_kernel_spmd(nc, in_maps, core_ids=list(range(N_CORES)))
    out = np.concatenate([res.results[i]["out"] for i in range(N_CORES)], axis=0)
    return out


# revision 16
# speedup vs baseline: 1.4605x; 1.1617x over previous
"""CQAttention Trainium2 kernel.

Math (per batch b):
  S = (C*w3) @ Q^T + (C@w1)[:,None] + (Q@w2)[None,:] (+bias, dropped: softmax-invariant)
  Sq = softmax over q of qmask-masked S ; Sc = softmax over c of cmask-masked S
  A = Sq@Q ; Bm = Sq @ (Sc^T @ C) ; out = [C | A | C*A | C*Bm]

Device algorithm. No max-subtraction (|S| < 1 so exp is safe). All rank-1
bias/mask factors are precomputed on host as multiplicative exp factors and
folded into matmul operands, so the device only computes:
  ST  = (Q*w3) @ C^T                [q, c]   (PE, fp32r)
  X   = exp(ST)                     [q, c]
  XTg = X^T * g[c]                  [c, q]   g = exp(C@w1 + cneg), fold in the
                                             PSUM->SBUF copy of the transpose
  T1  = XTg^T @ [C|1]               [q, d+2] numerator | W (col denominators)
  T1g = T1[:, :d] * (eb/W)[q]       [q, d]   eb = exp(Q@w2 + qneg)
  psA = X^T @ [Q*eb | eb]           [c, d+2] unnormalized A | Z (row denoms)
  psB = X^T @ T1g                   [c, d]   unnormalized Bm
  A = psA/Z ; CA = C*A ; CBm = C*psB/Z

Sharding: data-parallel over batch, 4 batches per core on 8 cores.
"""

import numpy as np

NEG_INF = -1e30
B_FULL, LC, LQ, D = 32, 1024, 128, 256
D2 = D + 2
N_CORES = 8
NB = B_FULL // N_CORES  # batches per core
KC = LC // 128  # c-tiles per batch (8)

_CACHE = {}


def _build_nc():
    import concourse.bacc as bacc
    import concourse.mybir as mybir
    from concourse import tile
    from concourse.masks import make_identity

    fp32 = mybir.dt.float32
    fp32r = mybir.dt.float32r
    MULT = mybir.AluOpType.mult
    EXP = mybir.ActivationFunctionType.Exp

    nc = bacc.Bacc("TRN2", target_bir_lowering=False, debug=False)

    C_d = nc.dram_tensor("Cp", [NB, LC, D2], fp32, kind="ExternalInput")
    qt3_d = nc.dram_tensor("qt3", [128, NB, 2, 128], fp32, kind="ExternalInput")
    qg_d = nc.dram_tensor("qg", [128, NB, D2], fp32, kind="ExternalInput")
    meta_d = nc.dram_tensor("meta", [128, NB, KC + 1], fp32, kind="ExternalInput")
    out_d = nc.dram_tensor("out", [NB, LC, 4 * D], fp32, kind="ExternalOutput")

    with tile.TileContext(nc) as tc:
        with (
            tc.tile_pool(name="const", bufs=1) as const,
            tc.tile_pool(name="cpool", bufs=NB) as p_c,
            tc.tile_pool(name="crpool", bufs=2) as p_cr,
            tc.tile_pool(name="ctpool", bufs=2) as p_ct,
            tc.tile_pool(name="xpool", bufs=2) as p_x,
            tc.tile_pool(name="xtpool", bufs=2) as p_xt,
            tc.tile_pool(name="t1gpool", bufs=2) as p_t1g,
            tc.tile_pool(name="smpool", bufs=6) as p_sm,
            tc.tile_pool(name="opool", bufs=6) as p_o,
            tc.tile_pool(name="pstr", bufs=3, space="PSUM") as ps_tr,
            tc.tile_pool(name="pst1", bufs=1, space="PSUM") as ps_t1,
            tc.tile_pool(name="psa", bufs=2, space="PSUM") as ps_a,
            tc.tile_pool(name="psb", bufs=2, space="PSUM") as ps_b,
        ):
            ident = const.tile([128, 128], fp32)
            make_identity(nc, ident)
            identr = const.tile([128, 128], fp32r)
            nc.vector.tensor_copy(identr, ident)

            # ---- hoisted input loads (first C tile first, so PE can start) ----
            C1s = [None] * NB
            C1s[0] = p_c.tile([128, KC, D2], fp32, tag="c", name="c1_first")
            nc.sync.dma_start(C1s[0], C_d.ap()[0].rearrange("(k p) d -> p k d", p=128))

            qt3f = const.tile([128, NB, 2, 128], fp32)
            nc.sync.dma_start(qt3f, qt3_d.ap())
            qgf = const.tile([128, NB, D2], fp32)
            nc.sync.dma_start(qgf, qg_d.ap())
            meta = const.tile([128, NB, KC + 1], fp32)
            nc.sync.dma_start(meta, meta_d.ap())

            for b in range(1, NB):
                C1s[b] = p_c.tile([128, KC, D2], fp32, tag="c", name="c1")
                nc.sync.dma_start(
                    C1s[b], C_d.ap()[b].rearrange("(k p) d -> p k d", p=128)
                )

            # one-time fp32r rounding of the shared matmul operands
            qt3 = const.tile([128, NB, 2, 128], fp32r)
            nc.vector.tensor_copy(qt3, qt3f)
            qg = const.tile([128, NB, D2], fp32r)
            nc.scalar.copy(qg, qgf)

            for b in range(NB):
                C1 = C1s[b]

                # ---- rounded [C|1] copy for the T1 rhs ----
                Cr = p_cr.tile([128, KC, D2], fp32r, tag="cr")
                nc.scalar.copy(Cr[:, 0 : KC // 2], C1[:, 0 : KC // 2])
                nc.vector.tensor_copy(Cr[:, KC // 2 : KC], C1[:, KC // 2 : KC])

                # ---- CT = C^T per d-chunk ----
                CT = p_ct.tile([128, 2, LC], fp32r, tag="ct")
                for dk in range(2):
                    for h in range(2):
                        pt = ps_tr.tile([128, 512], fp32, tag="pt")
                        for j in range(4):
                            k = h * 4 + j
                            nc.tensor.transpose(
                                pt[:, j * 128 : (j + 1) * 128],
                                C1[:, k, dk * 128 : (dk + 1) * 128],
                                ident,
                            )
                        dst = CT[:, dk, h * 512 : (h + 1) * 512]
                        if (dk * 2 + h) % 2 == 0:
                            nc.scalar.copy(dst, pt)
                        else:
                            nc.vector.tensor_copy(dst, pt)

                # ---- ST = (Q*w3) @ C^T ; X = exp(ST) ----
                X = p_x.tile([128, LC], fp32r, tag="x")
                for h in range(2):
                    st = ps_tr.tile([128, 512], fp32, tag="pt")
                    for dk in range(2):
                        nc.tensor.matmul(
                            st,
                            qt3[:, b, dk],
                            CT[:, dk, h * 512 : (h + 1) * 512],
                            start=(dk == 0),
                            stop=(dk == 1),
                        )
                    nc.scalar.activation(X[:, h * 512 : (h + 1) * 512], st, EXP)

                # ---- XTg = X^T * g (g folded into the PSUM->SBUF copy) ----
                XT = p_xt.tile([128, KC, 128], fp32r, tag="xt")
                for h in range(2):
                    pt = ps_tr.tile([128, 512], fp32r, tag="pt")
                    for j in range(4):
                        k = h * 4 + j
                        nc.tensor.transpose(
                            pt[:, j * 128 : (j + 1) * 128],
                            X[:, k * 128 : (k + 1) * 128],
                            identr,
                        )
                    for j in range(4):
                        k = h * 4 + j
                        src = pt[:, j * 128 : (j + 1) * 128]
                        gk = meta[:, b, k : k + 1]
                        if j % 2 == 0:
                            nc.scalar.mul(XT[:, k], src, gk)
                        else:
                            nc.vector.tensor_scalar_mul(XT[:, k], src, gk)

                # ---- T1 = XTg^T @ [C|1] ; T1g = T1 * (eb/W) ----
                t1 = ps_t1.tile([128, D2], fp32, tag="t1")
                for k in range(KC):
                    nc.tensor.matmul(
                        t1,
                        XT[:, k],
                        Cr[:, k],
                        start=(k == 0),
                        stop=(k == KC - 1),
                    )
                recipT = p_sm.tile([128, 1], fp32, tag="recipT")
                nc.vector.reciprocal(recipT, t1[:, D : D + 1])
                scal = p_sm.tile([128, 1], fp32, tag="scal")
                nc.vector.tensor_mul(scal, recipT, meta[:, b, KC : KC + 1])
                T1g = p_t1g.tile([128, D], fp32r, tag="t1g")
                nc.vector.tensor_scalar_mul(T1g, t1[:, 0:D], scal)

                # ---- per c-tile: psA / psB -> A / CA / CBm ----
                psAs = [None] * KC
                psAs[0] = ps_a.tile([128, D2], fp32, tag="psa", name="psa0")
                nc.tensor.matmul(psAs[0], X[:, 0:128], qg[:, b], start=True, stop=True)
                for k in range(KC):
                    kk = k % 2
                    if kk == 0:
                        osb = p_o.tile([128, 2, 3 * D], fp32, tag="osb")
                    if k + 1 < KC:
                        psAs[k + 1] = ps_a.tile([128, D2], fp32, tag="psa", name="psa")
                        nc.tensor.matmul(
                            psAs[k + 1],
                            X[:, (k + 1) * 128 : (k + 2) * 128],
                            qg[:, b],
                            start=True,
                            stop=True,
                        )
                    psA = psAs[k]
                    psB = ps_b.tile([128, D], fp32, tag="psb")
                    nc.tensor.matmul(
                        psB, X[:, k * 128 : (k + 1) * 128], T1g, start=True, stop=True
                    )

                    rr = p_sm.tile([128, 1], fp32, tag="rr")
                    nc.vector.reciprocal(rr, psA[:, D : D + 1])

                    # A = psA * rr  (ACT, per-partition scale)
                    nc.scalar.mul(osb[:, kk, 0:D], psA[:, 0:D], rr)
                    # CA = C * A  (GPSIMD; reads the extracted A from SBUF)
                    nc.gpsimd.tensor_mul(
                        osb[:, kk, D : 2 * D], C1[:, k, 0:D], osb[:, kk, 0:D]
                    )
                    # CBm = (psB * rr) * C  (DVE fused)
                    nc.vector.scalar_tensor_tensor(
                        osb[:, kk, 2 * D : 3 * D], psB, rr, C1[:, k, 0:D], MULT, MULT
                    )
                    if kk == 1:
                        nc.sync.dma_start(
                            out_d.ap()[
                                b, (k - 1) * 128 : (k + 1) * 128, D : 4 * D
                            ].rearrange("(k p) n -> p k n", p=128),
                            osb,
                        )
                        if k == 1:
                            # C passthrough store: always-ready FIFO filler
                            # between this batch's compute stores.
                            nc.sync.dma_start(
                                out_d.ap()[b, :, 0:D].rearrange(
                                    "(k p) d -> p k d", p=128
                                ),
                                C1[:, :, 0:D],
                            )

    nc.compile()
    return nc


def _get_nc():
    if "nc" not in _CACHE:
        _CACHE["nc"] = _build_nc()
    return _CACHE["nc"]


def _make_in_maps(C, Q, cmask, qmask, Wo_w):
    C = np.ascontiguousarray(C, dtype=np.float32)
    Q = np.ascontiguousarray(Q, dtype=np.float32)
    w = np.asarray(Wo_w, dtype=np.float32)
    w1, w2, w3 = w[:D], w[D : 2 * D], w[2 * D :]

    rc = (C @ w1).astype(np.float32)  # [B, Lc]
    rq = (Q @ w2).astype(np.float32)  # [B, Lq]
    cneg = ((1.0 - cmask.astype(np.float32)) * NEG_INF).astype(np.float32)
    qneg = ((1.0 - qmask.astype(np.float32)) * NEG_INF).astype(np.float32)
    with np.errstate(under="ignore", over="ignore"):
        g = np.exp(rc + cneg).astype(np.float32)  # [B, Lc]
        eb = np.exp(rq + qneg).astype(np.float32)  # [B, Lq]

    ones2 = np.ones((B_FULL, LC, 2), np.float32)
    Cp = np.concatenate([C, ones2], axis=2)  # [B, Lc, 258]

    QT3 = (C.dtype.type(1) * Q.transpose(0, 2, 1) * w3[None, :, None]).reshape(
        B_FULL, 2, 128, LQ
    )
    QT3 = QT3.transpose(0, 2, 1, 3)  # [B, 128(p), 2(dk), 128(q)]

    ebc = eb[:, :, None]
    Qg = np.concatenate([Q * ebc, ebc, ebc], axis=2)  # [B, 128, 258]

    gm = g.reshape(B_FULL, KC, 128).transpose(0, 2, 1)  # [B, 128, KC]
    meta = np.concatenate([gm, eb[:, :, None]], axis=2)  # [B, 128, KC+1]

    in_maps = []
    for i in range(N_CORES):
        sl = slice(i * NB, (i + 1) * NB)
        in_maps.append(
            {
                "Cp": np.ascontiguousarray(Cp[sl]),
                "qt3": np.ascontiguousarray(QT3[sl].transpose(1, 0, 2, 3)),
                "qg": np.ascontiguousarray(Qg[sl].transpose(1, 0, 2)),
                "meta": np.ascontiguousarray(meta[sl].transpose(1, 0, 2)),
            }
        )
    return in_maps


def kernel(C, Q, cmask, qmask, Wo_w, Wo_b):
    from concourse.bass_utils import run_bass_kernel_spmd

    nc = _get_nc()
    in_maps = _make_in_maps(C, Q, cmask, qmask, Wo_w)
    res = run_bass_kernel_spmd(nc, in_maps, core_ids=list(range(N_CORES)))
    out = np.concatenate([res.results[i]["out"] for i in range(N_CORES)], axis=0)
    return out


# revision 20
# speedup vs baseline: 1.4757x; 1.0104x over previous
"""CQAttention Trainium2 kernel.

Math (per batch b):
  S = (C*w3) @ Q^T + (C@w1)[:,None] + (Q@w2)[None,:] (+bias, dropped: softmax-invariant)
  Sq = softmax over q of qmask-masked S ; Sc = softmax over c of cmask-masked S
  A = Sq@Q ; Bm = Sq @ (Sc^T @ C) ; out = [C | A | C*A | C*Bm]

Device algorithm. No max-subtraction (|S| < 1 so exp is safe). All rank-1
bias/mask factors are precomputed on host as multiplicative exp factors and
folded into matmul operands, so the device only computes:
  ST  = (Q*w3) @ C^T                [q, c]   (PE, fp32r)
  X   = exp(ST)                     [q, c]
  XTg = X^T * g[c]                  [c, q]   g = exp(C@w1 + cneg), fold in the
                                             PSUM->SBUF copy of the transpose
  T1  = XTg^T @ [C|1]               [q, d+2] numerator | W (col denominators)
  T1g = T1[:, :d] * (eb/W)[q]       [q, d]   eb = exp(Q@w2 + qneg)
  psA = X^T @ [Q*eb | eb]           [c, d+2] unnormalized A | Z (row denoms)
  psB = X^T @ T1g                   [c, d]   unnormalized Bm
  A = psA/Z ; CA = C*A ; CBm = C*psB/Z

Sharding: data-parallel over batch, 4 batches per core on 8 cores.
"""

import numpy as np

NEG_INF = -1e30
B_FULL, LC, LQ, D = 32, 1024, 128, 256
D2 = D + 2
N_CORES = 8
NB = B_FULL // N_CORES  # batches per core
KC = LC // 128  # c-tiles per batch (8)

_CACHE = {}


def _build_nc():
    import concourse.bacc as bacc
    import concourse.mybir as mybir
    from concourse import tile
    from concourse.masks import make_identity

    fp32 = mybir.dt.float32
    fp32r = mybir.dt.float32r
    MULT = mybir.AluOpType.mult
    EXP = mybir.ActivationFunctionType.Exp

    nc = bacc.Bacc("TRN2", target_bir_lowering=False, debug=False)

    # Cp is host-relaid to partition-major [NB, 128(p), KC, D2] so each
    # SBUF partition line is one contiguous 8KB DRAM read.
    C_d = nc.dram_tensor("Cp", [NB, 128, KC, D2], fp32, kind="ExternalInput")
    qt3_d = nc.dram_tensor("qt3", [128, NB, 2, 128], fp32, kind="ExternalInput")
    qg_d = nc.dram_tensor("qg", [128, NB, D2], fp32, kind="ExternalInput")
    meta_d = nc.dram_tensor("meta", [128, NB, KC + 1], fp32, kind="ExternalInput")
    out_d = nc.dram_tensor("out", [NB, LC, 4 * D], fp32, kind="ExternalOutput")

    with tile.TileContext(nc) as tc:
        with (
            tc.tile_pool(name="const", bufs=1) as const,
            tc.tile_pool(name="cpool", bufs=NB) as p_c,
            tc.tile_pool(name="crpool", bufs=2) as p_cr,
            tc.tile_pool(name="ctpool", bufs=2) as p_ct,
            tc.tile_pool(name="xpool", bufs=2) as p_x,
            tc.tile_pool(name="xtpool", bufs=2) as p_xt,
            tc.tile_pool(name="t1gpool", bufs=2) as p_t1g,
            tc.tile_pool(name="smpool", bufs=6) as p_sm,
            tc.tile_pool(name="opool", bufs=6) as p_o,
            tc.tile_pool(name="pstr", bufs=3, space="PSUM") as ps_tr,
            tc.tile_pool(name="pst1", bufs=1, space="PSUM") as ps_t1,
            tc.tile_pool(name="psa", bufs=2, space="PSUM") as ps_a,
            tc.tile_pool(name="psb", bufs=2, space="PSUM") as ps_b,
        ):
            ident = const.tile([128, 128], fp32)
            make_identity(nc, ident)
            identr = const.tile([128, 128], fp32r)
            nc.vector.tensor_copy(identr, ident)

            # ---- hoisted input loads (first C tile first, so PE can start) ----
            C1s = [None] * NB
            C1s[0] = p_c.tile([128, KC, D2], fp32, tag="c", name="c1_first")
            nc.sync.dma_start(C1s[0], C_d.ap()[0])

            qt3f = const.tile([128, NB, 2, 128], fp32)
            nc.sync.dma_start(qt3f, qt3_d.ap())
            qgf = const.tile([128, NB, D2], fp32)
            nc.sync.dma_start(qgf, qg_d.ap())
            meta = const.tile([128, NB, KC + 1], fp32)
            nc.sync.dma_start(meta, meta_d.ap())

            for b in range(1, NB):
                C1s[b] = p_c.tile([128, KC, D2], fp32, tag="c", name="c1")
                nc.sync.dma_start(C1s[b], C_d.ap()[b])

            # one-time fp32r rounding of the shared matmul operands
            qt3 = const.tile([128, NB, 2, 128], fp32r)
            nc.vector.tensor_copy(qt3, qt3f)
            qg = const.tile([128, NB, D2], fp32r)
            nc.scalar.copy(qg, qgf)

            for b in range(NB):
                C1 = C1s[b]

                # ---- rounded [C|1] copy for the T1 rhs ----
                Cr = p_cr.tile([128, KC, D2], fp32r, tag="cr")
                nc.scalar.copy(Cr[:, 0 : KC // 2], C1[:, 0 : KC // 2])
                nc.vector.tensor_copy(Cr[:, KC // 2 : KC], C1[:, KC // 2 : KC])

                # ---- CT = C^T per d-chunk ----
                CT = p_ct.tile([128, 2, LC], fp32r, tag="ct")
                for dk in range(2):
                    for h in range(2):
                        pt = ps_tr.tile([128, 512], fp32, tag="pt")
                        for j in range(4):
                            k = h * 4 + j
                            nc.tensor.transpose(
                                pt[:, j * 128 : (j + 1) * 128],
                                C1[:, k, dk * 128 : (dk + 1) * 128],
                                ident,
                            )
                        dst = CT[:, dk, h * 512 : (h + 1) * 512]
                        if (dk * 2 + h) % 2 == 0:
                            nc.scalar.copy(dst, pt)
                        else:
                            nc.vector.tensor_copy(dst, pt)

                # ---- ST = (Q*w3) @ C^T ; X = exp(ST) ----
                X = p_x.tile([128, LC], fp32r, tag="x")
                for h in range(2):
                    st = ps_tr.tile([128, 512], fp32, tag="pt")
                    for dk in range(2):
                        nc.tensor.matmul(
                            st,
                            qt3[:, b, dk],
                            CT[:, dk, h * 512 : (h + 1) * 512],
                            start=(dk == 0),
                            stop=(dk == 1),
                        )
                    nc.scalar.activation(X[:, h * 512 : (h + 1) * 512], st, EXP)

                # ---- XTg = X^T * g (g folded into the PSUM->SBUF copy) ----
                XT = p_xt.tile([128, KC, 128], fp32r, tag="xt")
                for h in range(2):
                    pt = ps_tr.tile([128, 512], fp32r, tag="pt")
                    for j in range(4):
                        k = h * 4 + j
                        nc.tensor.transpose(
                            pt[:, j * 128 : (j + 1) * 128],
                            X[:, k * 128 : (k + 1) * 128],
                            identr,
                        )
                    for j in range(4):
                        k = h * 4 + j
                        src = pt[:, j * 128 : (j + 1) * 128]
                        gk = meta[:, b, k : k + 1]
                        if j % 2 == 0:
                            nc.scalar.mul(XT[:, k], src, gk)
                        else:
                            nc.vector.tensor_scalar_mul(XT[:, k], src, gk)

                # ---- T1 = XTg^T @ [C|1] ; T1g = T1 * (eb/W) ----
                t1 = ps_t1.tile([128, D2], fp32, tag="t1")
                for k in range(KC):
                    nc.tensor.matmul(
                        t1,
                        XT[:, k],
                        Cr[:, k],
                        start=(k == 0),
                        stop=(k == KC - 1),
                    )
                recipT = p_sm.tile([128, 1], fp32, tag="recipT")
                nc.vector.reciprocal(recipT, t1[:, D : D + 1])
                scal = p_sm.tile([128, 1], fp32, tag="scal")
                nc.vector.tensor_mul(scal, recipT, meta[:, b, KC : KC + 1])
                T1g = p_t1g.tile([128, D], fp32r, tag="t1g")
                nc.vector.tensor_scalar_mul(T1g, t1[:, 0:D], scal)

                # ---- per c-tile: psA / psB -> A / CA / CBm ----
                psAs = [None] * KC
                psAs[0] = ps_a.tile([128, D2], fp32, tag="psa", name="psa0")
                nc.tensor.matmul(psAs[0], X[:, 0:128], qg[:, b], start=True, stop=True)
                for k in range(KC):
                    kk = k % 2
                    if kk == 0:
                        osb = p_o.tile([128, 2, 3 * D], fp32, tag="osb")
                    if k + 1 < KC:
                        psAs[k + 1] = ps_a.tile([128, D2], fp32, tag="psa", name="psa")
                        nc.tensor.matmul(
                            psAs[k + 1],
                            X[:, (k + 1) * 128 : (k + 2) * 128],
                            qg[:, b],
                            start=True,
                            stop=True,
                        )
                    psA = psAs[k]
                    psB = ps_b.tile([128, D], fp32, tag="psb")
                    nc.tensor.matmul(
                        psB, X[:, k * 128 : (k + 1) * 128], T1g, start=True, stop=True
                    )

                    rr = p_sm.tile([128, 1], fp32, tag="rr")
                    nc.vector.reciprocal(rr, psA[:, D : D + 1])

                    # A = psA * rr  (ACT, per-partition scale)
                    nc.scalar.mul(osb[:, kk, 0:D], psA[:, 0:D], rr)
                    # CA = C * A  (GPSIMD; reads the extracted A from SBUF)
                    nc.gpsimd.tensor_mul(
                        osb[:, kk, D : 2 * D], C1[:, k, 0:D], osb[:, kk, 0:D]
                    )
                    # CBm = (psB * rr) * C  (DVE fused)
                    nc.vector.scalar_tensor_tensor(
                        osb[:, kk, 2 * D : 3 * D], psB, rr, C1[:, k, 0:D], MULT, MULT
                    )
                    if kk == 1:
                        nc.sync.dma_start(
                            out_d.ap()[
                                b, (k - 1) * 128 : (k + 1) * 128, D : 4 * D
                            ].rearrange("(k p) n -> p k n", p=128),
                            osb,
                        )
                        if k == 1:
                            # C passthrough store: always-ready FIFO filler
                            # between this batch's compute stores.
                            nc.sync.dma_start(
                                out_d.ap()[b, :, 0:D].rearrange(
                                    "(k p) d -> p k d", p=128
                                ),
                                C1[:, :, 0:D],
                            )

    nc.compile()
    return nc


def _get_nc():
    if "nc" not in _CACHE:
        _CACHE["nc"] = _build_nc()
    return _CACHE["nc"]


def _make_in_maps(C, Q, cmask, qmask, Wo_w):
    C = np.ascontiguousarray(C, dtype=np.float32)
    Q = np.ascontiguousarray(Q, dtype=np.float32)
    w = np.asarray(Wo_w, dtype=np.float32)
    w1, w2, w3 = w[:D], w[D : 2 * D], w[2 * D :]

    rc = (C @ w1).astype(np.float32)  # [B, Lc]
    rq = (Q @ w2).astype(np.float32)  # [B, Lq]
    cneg = ((1.0 - cmask.astype(np.float32)) * NEG_INF).astype(np.float32)
    qneg = ((1.0 - qmask.astype(np.float32)) * NEG_INF).astype(np.float32)
    with np.errstate(under="ignore", over="ignore"):
        g = np.exp(rc + cneg).astype(np.float32)  # [B, Lc]
        eb = np.exp(rq + qneg).astype(np.float32)  # [B, Lq]

    ones2 = np.ones((B_FULL, LC, 2), np.float32)
    Cp = np.concatenate([C, ones2], axis=2)  # [B, Lc, 258]
    # partition-major relayout: [B, KC, 128(p), D2] -> [B, 128(p), KC, D2]
    Cp = Cp.reshape(B_FULL, KC, 128, D2).transpose(0, 2, 1, 3)

    QT3 = (C.dtype.type(1) * Q.transpose(0, 2, 1) * w3[None, :, None]).reshape(
        B_FULL, 2, 128, LQ
    )
    QT3 = QT3.transpose(0, 2, 1, 3)  # [B, 128(p), 2(dk), 128(q)]

    ebc = eb[:, :, None]
    Qg = np.concatenate([Q * ebc, ebc, ebc], axis=2)  # [B, 128, 258]

    gm = g.reshape(B_FULL, KC, 128).transpose(0, 2, 1)  # [B, 128, KC]
    meta = np.concatenate([gm, eb[:, :, None]], axis=2)  # [B, 128, KC+1]

    in_maps = []
    for i in range(N_CORES):
        sl = slice(i * NB, (i + 1) * NB)
        in_maps.append(
            {
                "Cp": np.ascontiguousarray(Cp[sl]),
                "qt3": np.ascontiguousarray(QT3[sl].transpose(1, 0, 2, 3)),
                "qg": np.ascontiguousarray(Qg[sl].transpose(1, 0, 2)),
                "meta": np.ascontiguousarray(meta[sl].transpose(1, 0, 2)),
            }
        )
    return in_maps


def kernel(C, Q, cmask, qmask, Wo_w, Wo_b):
    from concourse.bass_utils import run_bass_kernel_spmd

    nc = _get_nc()
    in_maps = _make_in_maps(C, Q, cmask, qmask, Wo_w)
    res = run_bass_kernel_spmd(nc, in_maps, core_ids=list(range(N_CORES)))
    out = np.concatenate([res.results[i]["out"] for i in range(N_CORES)], axis=0)
    return out
